# revision 9
# baseline (speedup 1.0000x reference)
"""2-layer GAT on 8 Trainium2 NeuronCores (Bass/Tile, SPMD) — v2.

Sharding: destination nodes i are partitioned across the 8 cores (512 rows
each); each core computes softmax + aggregation over all N=4096 sources for
its slice, both layers.

v2 changes vs the collective baseline:
- Layer-1 projection g = x @ W1 is SHARDED: each core projects only its own
  512 source rows (8.2k PE cycles instead of 65k replicated) and the [N,1028]
  bf16 g_aug matrix is exchanged SBUF-to-SBUF via remote_dma_broadcast
  (XOR-relative all-8 dests, dynamic out slot = partition_id). This also
  removes 7/8 of the PSUM->SBUF copies that saturated DVE/ACT.
- The inter-layer AllGather collective (28.5us latency bubble) is replaced by
  the same RDMA broadcast (~1.5us).
- Cross-core sync: remote sems, with wait_ge attached to the first consumer
  instructions AFTER tile scheduling (the single-core scheduling pass would
  otherwise deadlock on sems only remote cores increment).

Score pipeline per (j-tile, head) unchanged (it is locally optimal):
  el = lrelu((s_rep + t_j) + nonadj*(-1e9))   (fused custom DVE op)
  p  = exp(el)                                (ACT, head-pair-wide)
  out[i,f], Z[i] += p^T @ [g | 1]             (PE, f32 PSUM accum)
"""

import numpy as np
import ml_dtypes

import concourse.bass as bass
import concourse.bacc as bacc
import concourse.mybir as mybir
import concourse.tile as tile
from concourse.bass_utils import run_bass_kernel_spmd

N = 4096
IN = 256
HID = 256
HEADS = 4
CLS = 64
SLOPE = 0.2
NCORES = 8
IS = N // NCORES          # 512 destination rows per core
ICHUNKS = IS // 128       # 4
JT = N // 128             # 32 source-node tiles
OWNT = 4                  # own j-tiles per core
GW = HEADS * (HID + 1)    # 1028 g_aug cols per j-tile
G2W = CLS + 2             # 66 g2_aug cols
NEG = -1.0e9

F32 = mybir.dt.float32
BF16 = mybir.dt.bfloat16
ADD = mybir.AluOpType.add
MULT = mybir.AluOpType.mult
AF = mybir.ActivationFunctionType

BF = ml_dtypes.bfloat16

# ---- custom fused DVE op: out = lrelu((in0 + s0) + in1 * s1) ----------------
import concourse.dve_ops as _dve_ops
from concourse.dve_spec import Spec as _Spec, Src0 as _Src0, Src1 as _Src1, \
    C0 as _C0, C1 as _C1, C2 as _C2, maxx as _maxx, lower as _dve_lower, \
    _has_src1
from concourse.dve_uop import DveOpSpec as _DveOpSpec


def _gat_edge2_ref(in0, in1, s0, s1, imm2):
    z = (in0.astype(np.float32) + s0) + in1.astype(np.float32) * s1
    return np.maximum(z, z * imm2).astype(np.float32)


def _register(name, spec):
    if name in _dve_ops._SUB_OPCODE_FOR_NAME:
        return next(o for o in _dve_ops.OPS if o.name == name)
    opcode = _dve_ops._CUSTOM_DVE_ROW_BASE + len(_dve_ops.OPS)
    assert opcode < 0x20
    shas = {}
    for ver in ("v3", "v4"):
        s = _DveOpSpec(name=name, opcode=opcode,
                       uops=_dve_lower(spec, ver=ver), rd1_en=_has_src1(spec))
        shas[ver] = s.sha(ver)
    op = _dve_ops.DveOp(name, spec, subdim=False, uops_sha=shas)
    _dve_ops.OPS.append(op)
    _dve_ops._SUB_OPCODE_FOR_NAME[name] = opcode
    _dve_ops.CUSTOM_DVE_SPECS[name] = spec
    return op


_z2 = (_Src0 + _C0) + _Src1 * _C1
GAT_EDGE2 = _register("GAT_EDGE2",
                      _Spec(body=_maxx(_z2, _z2 * _C2), reference=_gat_edge2_ref))

_NC_CACHE = None


def build(reps=1, collectives=True):
    nc = bacc.Bacc("TRN2", target_bir_lowering=False, debug=False,
                   num_devices=NCORES)

    xt = nc.dram_tensor("xt", [IN, N], BF16, kind="ExternalInput")
    w1 = nc.dram_tensor("w1", [IN, HEADS * HID], BF16, kind="ExternalInput")
    srep = nc.dram_tensor("srep", [128, HEADS * IS], BF16, kind="ExternalInput")
    t1 = nc.dram_tensor("t1", [N, HEADS], F32, kind="ExternalInput")
    maskt = nc.dram_tensor("maskt", [N, IS], mybir.dt.uint8, kind="ExternalInput")
    w2a = nc.dram_tensor("w2a", [HID, G2W], BF16, kind="ExternalInput")
    y = nc.dram_tensor("y", [CLS + 1, IS], F32, kind="ExternalOutput")
    gath2 = [nc.dram_tensor(f"gath2_{r}", [N, G2W], BF16,
                            kind="Internal", addr_space="Shared") for r in range(reps)]
    groups = [list(range(NCORES))]

    with tile.TileContext(nc) as tc:
        with (
            tc.tile_pool(name="sb", bufs=1) as sb,        # persistent tiles
            tc.tile_pool(name="wk", bufs=3) as wk,        # rotating work tiles
            tc.tile_pool(name="ps", bufs=8, space="PSUM") as ps,
            tc.tile_pool(name="dram", bufs=1, space="DRAM") as dram_pool,
        ):
            # ---- resident inputs (batched DMAs, DVE-critical first) ---------
            # t1: [4096,4] -> [128, j(32) x h(4)] in one DMA
            t1_sb = sb.tile([128, JT * HEADS], F32, tag="t1", name="t1s")
            nc.sync.dma_start(
                t1_sb[:].rearrange("p (j h) -> p j h", h=HEADS),
                t1.rearrange("(j p) h -> p j h", p=128))
            srep_sb = sb.tile([128, HEADS * IS], BF16, tag="srep", name="sreps")
            nc.sync.dma_start(srep_sb[:], srep[:, :])
            # masks: [4096,512] -> [128, j(32) x 512] in two DMAs (halves)
            mask_all = sb.tile([128, JT * IS], mybir.dt.uint8, tag="mk", name="mk")
            for half in range(2):
                jj = slice(half * 16, (half + 1) * 16)
                nc.sync.dma_start(
                    mask_all[:, half * 16 * IS:(half + 1) * 16 * IS]
                    .rearrange("p (j c) -> p j c", c=IS),
                    maskt.rearrange("(j p) c -> p j c", p=128)[:, jj, :])
            mask_sb = [mask_all[:, j * IS:(j + 1) * IS] for j in range(JT)]
            # xt: [256, 4096] -> [128, k(2) x 4096] in two column-halves
            xt_sb = sb.tile([128, 2 * N], BF16, tag="xt", name="xts")
            for half in range(2):
                cs = slice(half * (N // 2), (half + 1) * (N // 2))
                nc.sync.dma_start(
                    xt_sb[:].rearrange("p (k c) -> p k c", k=2)[:, :, cs],
                    xt.rearrange("(k p) c -> p k c", p=128)[:, :, cs])
            # w1: [256, 1024] -> [128, k(2) x 1024] one DMA
            w1_sb = sb.tile([128, 2 * HEADS * HID], BF16, tag="w1", name="w1s")
            nc.sync.dma_start(
                w1_sb[:].rearrange("p (k c) -> p k c", k=2),
                w1.rearrange("(k p) c -> p k c", p=128))
            w2a_sb = sb.tile([128, 2 * G2W], BF16, tag="w2a", name="w2as")
            nc.sync.dma_start(
                w2a_sb[:].rearrange("p (k c) -> p k c", k=2),
                w2a.rearrange("(k p) c -> p k c", p=128))

            # identity for small transposes (s2 broadcast)
            from concourse.masks import make_identity
            ident = sb.tile([128, 128], BF16, tag="ident", name="ident")
            make_identity(nc, ident[:])
            ones1 = sb.tile([1, 128], BF16, tag="ones1", name="ones1")
            nc.vector.memset(ones1[:], 1.0)

            g2gath = sb.tile([128, NCORES * OWNT * G2W], BF16, tag="g2g", name="g2g")

            for rep in range(reps):
                # ---- layer-1 projection: replicated over all source nodes ---
                g_sb = [sb.tile([128, GW], BF16, tag=f"g{j}", name=f"g{j}")
                        for j in range(JT)]
                for j in range(JT):
                    for nh in range(2):
                        pj = ps.tile([128, 512], F32, tag="ps", name="pj")
                        for k in range(2):
                            nc.tensor.matmul(
                                pj[:],
                                lhsT=xt_sb[:, k * N + j * 128:k * N + (j + 1) * 128],
                                rhs=w1_sb[:, k * 1024 + nh * 512:k * 1024 + (nh + 1) * 512],
                                start=(k == 0), stop=(k == 1),
                            )
                        dst = g_sb[j][:, 2 * nh * 257:2 * nh * 257 + 514]
                        dst = dst.rearrange("p (b c) -> p b c", c=257)[:, :, 0:HID]
                        srcv = pj[:].rearrange("p (b c) -> p b c", c=HID)
                        if nh == 0:
                            nc.vector.tensor_copy(dst, srcv)
                        else:
                            nc.scalar.copy(dst, srcv)
                    for h in range(HEADS):
                        nc.vector.memset(
                            g_sb[j][:, h * 257 + HID:h * 257 + HID + 1], 1.0)

                def g1(j):           # g_aug tile for source j-tile J
                    return g_sb[j][:]

                # ---- layer-1 attention --------------------------------------
                contrib = {}
                for hp in range(2):                       # head pairs
                    heads = (2 * hp, 2 * hp + 1)
                    agg = {}
                    for h in heads:
                        for m in range(ICHUNKS):
                            agg[h, m] = ps.tile([128, HID + 1], F32, tag="ps",
                                                name=f"agg{h}_{m}")
                    for j in range(JT):
                        el = wk.tile([128, 2 * IS], BF16, tag="el", name="el", bufs=12)
                        for hi, h in enumerate(heads):
                            nc.vector._custom_dve(
                                GAT_EDGE2,
                                out=el[:, hi * IS:(hi + 1) * IS],
                                in0=srep_sb[:, h * IS:(h + 1) * IS],
                                in1=mask_sb[j],
                                s0=t1_sb[:, j * HEADS + h:j * HEADS + h + 1],
                                s1=NEG,
                                imm2=SLOPE,
                            )
                        p = wk.tile([128, 2 * IS], BF16, tag="p", name="p", bufs=12)
                        nc.scalar.activation(p[:], el[:], AF.Exp)
                        for hi, h in enumerate(heads):
                            for m in range(ICHUNKS):
                                nc.tensor.matmul(
                                    agg[h, m][:],
                                    lhsT=p[:, hi * IS + m * 128:hi * IS + (m + 1) * 128],
                                    rhs=g1(j)[:, h * (HID + 1):(h + 1) * (HID + 1)],
                                    start=(j == 0), stop=(j == JT - 1),
                                )
                    # normalize: contrib = agg / Z (head mean folded into W2)
                    for h in heads:
                        for m in range(ICHUNKS):
                            rz = wk.tile([128, 1], F32, tag="rz", name="rz")
                            nc.vector.reciprocal(rz[:], agg[h, m][:, HID:HID + 1])
                            ct = sb.tile([128, HID], F32, tag=f"ct{h}_{m}", name=f"ct{h}_{m}")
                            nc.scalar.activation(ct[:], agg[h, m][:, 0:HID],
                                                 AF.Copy, bias=0.0, scale=rz[:])
                            contrib[h, m] = ct

                # ---- head mean + ELU + g2_aug -------------------------------
                ht_sb = [sb.tile([128, IS], BF16, tag=f"ht{k}", name=f"ht{k}") for k in range(2)]
                s2own = sb.tile([128, ICHUNKS], F32, tag="s2own", name="s2own")
                g2own = sb.tile([128, OWNT * G2W], BF16, tag="g2own", name="g2own")
                for m in range(ICHUNKS):
                    a0 = wk.tile([128, HID], F32, tag="ha", name="ha")
                    nc.gpsimd.tensor_tensor(a0[:], contrib[0, m][:], contrib[1, m][:], ADD)
                    a1 = wk.tile([128, HID], F32, tag="hb", name="hb")
                    nc.gpsimd.tensor_tensor(a1[:], contrib[2, m][:], contrib[3, m][:], ADD)
                    hm = wk.tile([128, HID], F32, tag="hm", name="hm")
                    nc.vector.tensor_tensor(hm[:], a0[:], a1[:], ADD)
                    # ELU on hm/4: r = relu(hm/4); u = exp(hm/4 - r); helu = (r-1)+u
                    r = wk.tile([128, HID], F32, tag="hr", name="hr")
                    nc.scalar.activation(r[:], hm[:], AF.Relu, bias=0.0, scale=0.25)
                    mn = wk.tile([128, HID], F32, tag="hn", name="hn")
                    nc.vector.scalar_tensor_tensor(
                        out=mn[:], in0=hm[:], scalar=0.25, in1=r[:],
                        op0=MULT, op1=mybir.AluOpType.subtract)
                    u = wk.tile([128, HID], F32, tag="hu", name="hu")
                    nc.scalar.activation(u[:], mn[:], AF.Exp)
                    helu = wk.tile([128, HID], BF16, tag="helu", name="helu")
                    nc.vector.scalar_tensor_tensor(
                        out=helu[:], in0=r[:], scalar=-1.0, in1=u[:], op0=ADD, op1=ADD)
                    # transpose helu into ht_sb (layer-2 lhsT)
                    for k in range(2):
                        pt = ps.tile([128, 128], BF16, tag="ps", name="pt")
                        nc.tensor.transpose(pt[:], helu[:, k * 128:(k + 1) * 128], ident[:])
                        nc.vector.tensor_copy(ht_sb[k][:, m * 128:(m + 1) * 128], pt[:])
                for m in range(ICHUNKS):
                    pg = ps.tile([128, G2W], F32, tag="ps", name="pg")
                    for k in range(2):
                        nc.tensor.matmul(
                            pg[:], lhsT=ht_sb[k][:, m * 128:(m + 1) * 128],
                            rhs=w2a_sb[:, k * G2W:(k + 1) * G2W],
                            start=(k == 0), stop=(k == 1),
                        )
                    dst = g2own[:, m * G2W:(m + 1) * G2W]
                    nc.vector.tensor_copy(dst[:, 0:CLS], pg[:, 0:CLS])
                    nc.vector.memset(dst[:, CLS:CLS + 1], 1.0)
                    nc.vector.tensor_copy(dst[:, CLS + 1:CLS + 2], pg[:, CLS:CLS + 1])
                    nc.vector.tensor_copy(s2own[:, m:m + 1], pg[:, CLS + 1:CLS + 2])
                # g2 exchange: DRAM bounce -> AllGather -> one strided read
                bounce2 = dram_pool.tile([IS, G2W], BF16, tag="b2", name="b2")
                nc.sync.dma_start(
                    bounce2[:].rearrange("(m p) c -> p m c", p=128),
                    g2own[:].rearrange("p (m c) -> p m c", c=G2W))
                nc.gpsimd.collective_compute(
                    "AllGather", mybir.AluOpType.bypass, replica_groups=groups,
                    ins=[bounce2[:, :]], outs=[gath2[rep][:, :]],
                )
                nc.sync.dma_start(
                    g2gath[:].rearrange("p (j c) -> p j c", c=G2W),
                    gath2[rep].rearrange("(j p) c -> p j c", p=128))

                def g2(j):           # g2_aug tile for j-tile J: [128, 66]
                    return g2gath[:, j * G2W:(j + 1) * G2W]

                # t2 row: strided gather of col 65 from every tile
                t2_sb = sb.tile([128, JT], F32, tag="t2", name="t2s")
                t2src = g2gath[:].rearrange("p (j c) -> p j c", c=G2W)[:, :, CLS + 1]
                nc.vector.tensor_copy(t2_sb[:], t2src)

                # ---- s2 broadcast: [512] column -> [128, 512] row-replicated
                s2bf = wk.tile([128, ICHUNKS], BF16, tag="s2bf", name="s2bf")
                nc.vector.tensor_copy(s2bf[:], s2own[:])
                pt2 = ps.tile([1, IS], BF16, tag="ps", name="pt2")
                for m in range(ICHUNKS):
                    nc.tensor.transpose(
                        pt2[0:1, m * 128:(m + 1) * 128], s2bf[:, m:m + 1], ident[:])
                s2t = sb.tile([1, IS], BF16, tag="s2t", name="s2t")
                nc.vector.tensor_copy(s2t[:], pt2[:])
                pr = ps.tile([128, IS], F32, tag="ps", name="pr")
                nc.tensor.matmul(pr[:], lhsT=ones1[:], rhs=s2t[:], start=True, stop=True)
                s2rep = sb.tile([128, IS], BF16, tag="s2rep", name="s2rep")
                nc.vector.tensor_copy(s2rep[:], pr[:])

                # ---- layer-2 attention --------------------------------------
                agg2t = ps.tile([CLS + 1, 512], F32, tag="ps", name="agg2t")
                for jp in range(JT // 2):
                    el2 = wk.tile([128, 2 * IS], BF16, tag="el", name="el2", bufs=12)
                    for d in range(2):
                        j = 2 * jp + d
                        nc.vector._custom_dve(
                            GAT_EDGE2, out=el2[:, d * IS:(d + 1) * IS], in0=s2rep[:],
                            in1=mask_sb[j], s0=t2_sb[:, j:j + 1], s1=NEG, imm2=SLOPE)
                    p2 = wk.tile([128, 2 * IS], BF16, tag="p", name="p2", bufs=12)
                    nc.scalar.activation(p2[:], el2[:], AF.Exp)
                    for d in range(2):
                        j = 2 * jp + d
                        nc.tensor.matmul(
                            agg2t[:], lhsT=g2(j)[:, 0:CLS + 1],
                            rhs=p2[:, d * IS:(d + 1) * IS],
                            start=(j == 0), stop=(j == JT - 1),
                        )
                yt_sb = wk.tile([CLS + 1, 512], F32, tag="yt", name="yt")
                nc.scalar.copy(yt_sb[:], agg2t[:])
                nc.sync.dma_start(y[:, :], yt_sb[:])

    nc.compile()
    return nc


def _get_nc():
    global _NC_CACHE
    if _NC_CACHE is None:
        _NC_CACHE = build()
    return _NC_CACHE


def make_in_maps(x, adj_mat, W1, a1_src, a1_dst, W2, a2_src, a2_dst):
    x = np.asarray(x, dtype=np.float32)
    adj = np.asarray(adj_mat, dtype=bool)
    W1 = np.asarray(W1, dtype=np.float32)
    a1_src = np.asarray(a1_src, dtype=np.float32)
    a1_dst = np.asarray(a1_dst, dtype=np.float32)
    W2 = np.asarray(W2, dtype=np.float32)
    a2_src = np.asarray(a2_src, dtype=np.float32)
    a2_dst = np.asarray(a2_dst, dtype=np.float32)

    # host-side tiny precomputation (O(N*IN) matmuls with 8-col outputs)
    W1r = W1.astype(np.float64).reshape(IN, HEADS, HID)
    w1s = np.einsum("khf,f->kh", W1r, a1_src.astype(np.float64))
    w1d = np.einsum("khf,f->kh", W1r, a1_dst.astype(np.float64))
    xd = x.astype(np.float64)
    s1 = (xd @ w1s).astype(np.float32)          # [N, HEADS]
    t1 = (xd @ w1d).astype(np.float32)          # [N, HEADS]
    # W2 pre-scaled by 1/HEADS (head mean); aug cols: t2 | s2
    w2aug = np.concatenate(
        [W2, (W2.astype(np.float64) @ a2_dst.astype(np.float64))[:, None].astype(np.float32),
         (W2.astype(np.float64) @ a2_src.astype(np.float64))[:, None].astype(np.float32)],
        axis=1,
    )                                            # [HID, CLS+2]: g2 | t2 | s2
    mask_neg = (~adj).T.astype(np.uint8)                          # [N(j), N(i)]
    xt_all = np.ascontiguousarray(x.T).astype(BF)                  # [IN, N]
    w1_bf = W1.astype(BF)
    w2a_bf = w2aug.astype(BF)

    in_maps = []
    for c in range(NCORES):
        isl = slice(c * IS, (c + 1) * IS)
        srep_c = np.broadcast_to(
            np.ascontiguousarray(s1[isl].T).reshape(1, HEADS * IS), (128, HEADS * IS)
        ).astype(BF)
        in_maps.append({
            "xt": xt_all,
            "w1": w1_bf,
            "srep": np.ascontiguousarray(srep_c),
            "t1": t1,
            "maskt": np.ascontiguousarray(mask_neg[:, isl]),
            "w2a": w2a_bf,
        })

    return in_maps


def assemble_output(results):
    outs = []
    for c in range(NCORES):
        raw = results[c]["y"]        # [CLS+1, IS]: rows 0:CLS unnorm, row CLS = Z
        outs.append((raw[0:CLS] / raw[CLS:CLS + 1]).T)
    return np.concatenate(outs, axis=0).astype(np.float32)


def kernel(x, adj_mat, W1, a1_src, a1_dst, W2, a2_src, a2_dst):
    in_maps = make_in_maps(x, adj_mat, W1, a1_src, a1_dst, W2, a2_src, a2_dst)
    global _last_in_maps
    _last_in_maps = in_maps
    nc = _get_nc()
    res = run_bass_kernel_spmd(nc, in_maps, core_ids=list(range(NCORES)))
    return assemble_output(res.results)


# revision 11
# speedup vs baseline: 16.1447x; 16.1447x over previous
"""2-layer GAT on 8 Trainium2 NeuronCores (Bass/Tile, SPMD) — v3.

Sharding: destination nodes i are partitioned across the 8 cores (512 rows
each); each core computes softmax + aggregation over all N=4096 sources for
its slice, both layers. The layer-1 projection g = x @ W1 is computed
replicated on every core (remote DMA is rejected by this runtime, and an
8.4MB AllGather costs far more than the 27us of PE time). The only collective
is the small inter-layer AllGather of g2_aug = elu(h) @ [W2|t2|s2].

v3 changes vs the original baseline:
- All inputs load through 8 batched multi-dim-AP DMAs instead of 72 small
  ones (the 565ns/DMA SP sequencer serialization made el/p start ~18us late).
- The gathered g2 comes back as ONE strided DMA into a [128, 32*66] SBUF
  tile; t2 extraction is one strided tensor_copy instead of 32.
- Head-mean adds run on the idle GPSIMD engine; yt drain on ACT.
- A version-stamped "ver" output busts interface-keyed NEFF caches (the
  neuronxcc module hash ignores the custom call's embedded BIR, so kernels
  with identical I/O would silently reuse a stale NEFF) and proves which
  build executed.

Score pipeline per (j-tile, head) (engine-optimal per the cost model):
  el = lrelu((s_rep + t_j) + nonadj*(-1e9))   (fused custom DVE op, 1x mode)
  p  = exp(el)                                (ACT, head-pair-wide)
  out[i,f], Z[i] += p^T @ [g | 1]             (PE, f32 PSUM accum, 2 head-pair
                                               passes over j: PSUM fits 8x
                                               [128,257] f32 accumulators)
then out/Z per head (ACT Copy with scale=1/Z), head mean + ELU (scale 0.25
inside the ELU), layer 2 with g2 stationary (out^T in PSUM); the final
divide-by-Z and transpose happen on host from the [CLS+1, 512] raw slices.
"""

import numpy as np
import ml_dtypes

import concourse.bass as bass
import concourse.bacc as bacc
import concourse.mybir as mybir
import concourse.tile as tile
from concourse.bass_utils import run_bass_kernel_spmd

N = 4096
IN = 256
HID = 256
HEADS = 4
CLS = 64
SLOPE = 0.2
NCORES = 8
IS = N // NCORES          # 512 destination rows per core
ICHUNKS = IS // 128       # 4
JT = N // 128             # 32 source-node tiles
OWNT = 4                  # own j-tiles per core
GW = HEADS * (HID + 1)    # 1028 g_aug cols per j-tile
G2W = CLS + 2             # 66 g2_aug cols
NEG = -1.0e9

F32 = mybir.dt.float32
BF16 = mybir.dt.bfloat16
ADD = mybir.AluOpType.add
MULT = mybir.AluOpType.mult
AF = mybir.ActivationFunctionType

BF = ml_dtypes.bfloat16

# ---- custom fused DVE op: out = lrelu((in0 + s0) + in1 * s1) ----------------
import concourse.dve_ops as _dve_ops
from concourse.dve_spec import Spec as _Spec, Src0 as _Src0, Src1 as _Src1, \
    C0 as _C0, C1 as _C1, C2 as _C2, maxx as _maxx, lower as _dve_lower, \
    _has_src1
from concourse.dve_uop import DveOpSpec as _DveOpSpec


def _gat_edge2_ref(in0, in1, s0, s1, imm2):
    z = (in0.astype(np.float32) + s0) + in1.astype(np.float32) * s1
    return np.maximum(z, z * imm2).astype(np.float32)


def _register(name, spec):
    if name in _dve_ops._SUB_OPCODE_FOR_NAME:
        return next(o for o in _dve_ops.OPS if o.name == name)
    opcode = _dve_ops._CUSTOM_DVE_ROW_BASE + len(_dve_ops.OPS)
    assert opcode < 0x20
    shas = {}
    for ver in ("v3", "v4"):
        s = _DveOpSpec(name=name, opcode=opcode,
                       uops=_dve_lower(spec, ver=ver), rd1_en=_has_src1(spec))
        shas[ver] = s.sha(ver)
    op = _dve_ops.DveOp(name, spec, subdim=False, uops_sha=shas)
    _dve_ops.OPS.append(op)
    _dve_ops._SUB_OPCODE_FOR_NAME[name] = opcode
    _dve_ops.CUSTOM_DVE_SPECS[name] = spec
    return op


_z2 = (_Src0 + _C0) + _Src1 * _C1
GAT_EDGE2 = _register("GAT_EDGE2",
                      _Spec(body=_maxx(_z2, _z2 * _C2), reference=_gat_edge2_ref))

KVERSION = 3          # bump on every kernel change (stamped into "ver" output)
_NC_CACHE = None


def build(reps=1, collectives=True):
    nc = bacc.Bacc("TRN2", target_bir_lowering=False, debug=False,
                   num_devices=NCORES)

    xt = nc.dram_tensor("xt", [IN, N], BF16, kind="ExternalInput")
    w1 = nc.dram_tensor("w1", [IN, HEADS * HID], BF16, kind="ExternalInput")
    srep = nc.dram_tensor("srep", [128, HEADS * IS], BF16, kind="ExternalInput")
    t1 = nc.dram_tensor("t1", [N, HEADS], F32, kind="ExternalInput")
    maskt = nc.dram_tensor("maskt", [N, IS], mybir.dt.uint8, kind="ExternalInput")
    w2a = nc.dram_tensor("w2a", [HID, G2W], BF16, kind="ExternalInput")
    y = nc.dram_tensor("y", [CLS + 1, IS], F32, kind="ExternalOutput")
    ver = nc.dram_tensor("ver", [1, 2], F32, kind="ExternalOutput")
    gath2 = [nc.dram_tensor(f"gath2_{r}", [N, G2W], BF16,
                            kind="Internal", addr_space="Shared") for r in range(reps)]
    groups = [list(range(NCORES))]

    with tile.TileContext(nc) as tc:
        with (
            tc.tile_pool(name="sb", bufs=1) as sb,        # persistent tiles
            tc.tile_pool(name="wk", bufs=3) as wk,        # rotating work tiles
            tc.tile_pool(name="ps", bufs=8, space="PSUM") as ps,
            tc.tile_pool(name="dram", bufs=1, space="DRAM") as dram_pool,
        ):
            # ---- resident inputs (batched DMAs, DVE-critical first) ---------
            # t1: [4096,4] -> [128, j(32) x h(4)] in one DMA
            t1_sb = sb.tile([128, JT * HEADS], F32, tag="t1", name="t1s")
            nc.sync.dma_start(
                t1_sb[:].rearrange("p (j h) -> p j h", h=HEADS),
                t1.rearrange("(j p) h -> p j h", p=128))
            srep_sb = sb.tile([128, HEADS * IS], BF16, tag="srep", name="sreps")
            nc.sync.dma_start(srep_sb[:], srep[:, :])
            # masks: [4096,512] -> [128, j(32) x 512] in two DMAs (halves)
            mask_all = sb.tile([128, JT * IS], mybir.dt.uint8, tag="mk", name="mk")
            for half in range(2):
                jj = slice(half * 16, (half + 1) * 16)
                nc.sync.dma_start(
                    mask_all[:, half * 16 * IS:(half + 1) * 16 * IS]
                    .rearrange("p (j c) -> p j c", c=IS),
                    maskt.rearrange("(j p) c -> p j c", p=128)[:, jj, :])
            mask_sb = [mask_all[:, j * IS:(j + 1) * IS] for j in range(JT)]
            # xt: [256, 4096] -> [128, k(2) x 4096] in two column-halves
            xt_sb = sb.tile([128, 2 * N], BF16, tag="xt", name="xts")
            for half in range(2):
                cs = slice(half * (N // 2), (half + 1) * (N // 2))
                nc.sync.dma_start(
                    xt_sb[:].rearrange("p (k c) -> p k c", k=2)[:, :, cs],
                    xt.rearrange("(k p) c -> p k c", p=128)[:, :, cs])
            # w1: [256, 1024] -> [128, k(2) x 1024] one DMA
            w1_sb = sb.tile([128, 2 * HEADS * HID], BF16, tag="w1", name="w1s")
            nc.sync.dma_start(
                w1_sb[:].rearrange("p (k c) -> p k c", k=2),
                w1.rearrange("(k p) c -> p k c", p=128))
            w2a_sb = sb.tile([128, 2 * G2W], BF16, tag="w2a", name="w2as")
            nc.sync.dma_start(
                w2a_sb[:].rearrange("p (k c) -> p k c", k=2),
                w2a.rearrange("(k p) c -> p k c", p=128))

            # identity for small transposes (s2 broadcast)
            from concourse.masks import make_identity
            ident = sb.tile([128, 128], BF16, tag="ident", name="ident")
            make_identity(nc, ident[:])
            ones1 = sb.tile([1, 128], BF16, tag="ones1", name="ones1")
            nc.vector.memset(ones1[:], 1.0)

            g2gath = sb.tile([128, NCORES * OWNT * G2W], BF16, tag="g2g", name="g2g")
            vt = sb.tile([1, 2], F32, tag="vt", name="vt")
            nc.vector.memset(vt[:], float(KVERSION * 1000 + reps))
            nc.sync.dma_start(ver[:, :], vt[:])

            for rep in range(reps):
                # ---- layer-1 projection: replicated over all source nodes ---
                g_sb = [sb.tile([128, GW], BF16, tag=f"g{j}", name=f"g{j}")
                        for j in range(JT)]
                for j in range(JT):
                    for nh in range(2):
                        pj = ps.tile([128, 512], F32, tag="ps", name="pj")
                        for k in range(2):
                            nc.tensor.matmul(
                                pj[:],
                                lhsT=xt_sb[:, k * N + j * 128:k * N + (j + 1) * 128],
                                rhs=w1_sb[:, k * 1024 + nh * 512:k * 1024 + (nh + 1) * 512],
                                start=(k == 0), stop=(k == 1),
                            )
                        dst = g_sb[j][:, 2 * nh * 257:2 * nh * 257 + 514]
                        dst = dst.rearrange("p (b c) -> p b c", c=257)[:, :, 0:HID]
                        srcv = pj[:].rearrange("p (b c) -> p b c", c=HID)
                        if nh == 0:
                            nc.vector.tensor_copy(dst, srcv)
                        else:
                            nc.scalar.copy(dst, srcv)
                    for h in range(HEADS):
                        nc.vector.memset(
                            g_sb[j][:, h * 257 + HID:h * 257 + HID + 1], 1.0)

                def g1(j):           # g_aug tile for source j-tile J
                    return g_sb[j][:]

                # ---- layer-1 attention --------------------------------------
                contrib = {}
                for hp in range(2):                       # head pairs
                    heads = (2 * hp, 2 * hp + 1)
                    agg = {}
                    for h in heads:
                        for m in range(ICHUNKS):
                            agg[h, m] = ps.tile([128, HID + 1], F32, tag="ps",
                                                name=f"agg{h}_{m}")
                    for j in range(JT):
                        el = wk.tile([128, 2 * IS], BF16, tag="el", name="el", bufs=12)
                        for hi, h in enumerate(heads):
                            nc.vector._custom_dve(
                                GAT_EDGE2,
                                out=el[:, hi * IS:(hi + 1) * IS],
                                in0=srep_sb[:, h * IS:(h + 1) * IS],
                                in1=mask_sb[j],
                                s0=t1_sb[:, j * HEADS + h:j * HEADS + h + 1],
                                s1=NEG,
                                imm2=SLOPE,
                            )
                        p = wk.tile([128, 2 * IS], BF16, tag="p", name="p", bufs=12)
                        nc.scalar.activation(p[:], el[:], AF.Exp)
                        for hi, h in enumerate(heads):
                            for m in range(ICHUNKS):
                                nc.tensor.matmul(
                                    agg[h, m][:],
                                    lhsT=p[:, hi * IS + m * 128:hi * IS + (m + 1) * 128],
                                    rhs=g1(j)[:, h * (HID + 1):(h + 1) * (HID + 1)],
                                    start=(j == 0), stop=(j == JT - 1),
                                )
                    # normalize: contrib = agg / Z (head mean folded into W2)
                    for h in heads:
                        for m in range(ICHUNKS):
                            rz = wk.tile([128, 1], F32, tag="rz", name="rz")
                            nc.vector.reciprocal(rz[:], agg[h, m][:, HID:HID + 1])
                            ct = sb.tile([128, HID], F32, tag=f"ct{h}_{m}", name=f"ct{h}_{m}")
                            nc.scalar.activation(ct[:], agg[h, m][:, 0:HID],
                                                 AF.Copy, bias=0.0, scale=rz[:])
                            contrib[h, m] = ct

                # ---- head mean + ELU + g2_aug -------------------------------
                ht_sb = [sb.tile([128, IS], BF16, tag=f"ht{k}", name=f"ht{k}") for k in range(2)]
                s2own = sb.tile([128, ICHUNKS], F32, tag="s2own", name="s2own")
                g2own = sb.tile([128, OWNT * G2W], BF16, tag="g2own", name="g2own")
                for m in range(ICHUNKS):
                    a0 = wk.tile([128, HID], F32, tag="ha", name="ha")
                    nc.gpsimd.tensor_tensor(a0[:], contrib[0, m][:], contrib[1, m][:], ADD)
                    a1 = wk.tile([128, HID], F32, tag="hb", name="hb")
                    nc.gpsimd.tensor_tensor(a1[:], contrib[2, m][:], contrib[3, m][:], ADD)
                    hm = wk.tile([128, HID], F32, tag="hm", name="hm")
                    nc.vector.tensor_tensor(hm[:], a0[:], a1[:], ADD)
                    # ELU on hm/4: r = relu(hm/4); u = exp(hm/4 - r); helu = (r-1)+u
                    r = wk.tile([128, HID], F32, tag="hr", name="hr")
                    nc.scalar.activation(r[:], hm[:], AF.Relu, bias=0.0, scale=0.25)
                    mn = wk.tile([128, HID], F32, tag="hn", name="hn")
                    nc.vector.scalar_tensor_tensor(
                        out=mn[:], in0=hm[:], scalar=0.25, in1=r[:],
                        op0=MULT, op1=mybir.AluOpType.subtract)
                    u = wk.tile([128, HID], F32, tag="hu", name="hu")
                    nc.scalar.activation(u[:], mn[:], AF.Exp)
                    helu = wk.tile([128, HID], BF16, tag="helu", name="helu")
                    nc.vector.scalar_tensor_tensor(
                        out=helu[:], in0=r[:], scalar=-1.0, in1=u[:], op0=ADD, op1=ADD)
                    # transpose helu into ht_sb (layer-2 lhsT)
                    for k in range(2):
                        pt = ps.tile([128, 128], BF16, tag="ps", name="pt")
                        nc.tensor.transpose(pt[:], helu[:, k * 128:(k + 1) * 128], ident[:])
                        nc.vector.tensor_copy(ht_sb[k][:, m * 128:(m + 1) * 128], pt[:])
                for m in range(ICHUNKS):
                    pg = ps.tile([128, G2W], F32, tag="ps", name="pg")
                    for k in range(2):
                        nc.tensor.matmul(
                            pg[:], lhsT=ht_sb[k][:, m * 128:(m + 1) * 128],
                            rhs=w2a_sb[:, k * G2W:(k + 1) * G2W],
                            start=(k == 0), stop=(k == 1),
                        )
                    dst = g2own[:, m * G2W:(m + 1) * G2W]
                    nc.vector.tensor_copy(dst[:, 0:CLS], pg[:, 0:CLS])
                    nc.vector.memset(dst[:, CLS:CLS + 1], 1.0)
                    nc.vector.tensor_copy(dst[:, CLS + 1:CLS + 2], pg[:, CLS:CLS + 1])
                    nc.vector.tensor_copy(s2own[:, m:m + 1], pg[:, CLS + 1:CLS + 2])
                # g2 exchange: DRAM bounce -> AllGather -> one strided read
                bounce2 = dram_pool.tile([IS, G2W], BF16, tag="b2", name="b2")
                nc.sync.dma_start(
                    bounce2[:].rearrange("(m p) c -> p m c", p=128),
                    g2own[:].rearrange("p (m c) -> p m c", c=G2W))
                nc.gpsimd.collective_compute(
                    "AllGather", mybir.AluOpType.bypass, replica_groups=groups,
                    ins=[bounce2[:, :]], outs=[gath2[rep][:, :]],
                )
                nc.sync.dma_start(
                    g2gath[:].rearrange("p (j c) -> p j c", c=G2W),
                    gath2[rep].rearrange("(j p) c -> p j c", p=128))

                def g2(j):           # g2_aug tile for j-tile J: [128, 66]
                    return g2gath[:, j * G2W:(j + 1) * G2W]

                # t2 row: strided gather of col 65 from every tile
                t2_sb = sb.tile([128, JT], F32, tag="t2", name="t2s")
                t2src = g2gath[:].rearrange("p (j c) -> p j c", c=G2W)[:, :, CLS + 1]
                nc.vector.tensor_copy(t2_sb[:], t2src)

                # ---- s2 broadcast: [512] column -> [128, 512] row-replicated
                s2bf = wk.tile([128, ICHUNKS], BF16, tag="s2bf", name="s2bf")
                nc.vector.tensor_copy(s2bf[:], s2own[:])
                pt2 = ps.tile([1, IS], BF16, tag="ps", name="pt2")
                for m in range(ICHUNKS):
                    nc.tensor.transpose(
                        pt2[0:1, m * 128:(m + 1) * 128], s2bf[:, m:m + 1], ident[:])
                s2t = sb.tile([1, IS], BF16, tag="s2t", name="s2t")
                nc.vector.tensor_copy(s2t[:], pt2[:])
                pr = ps.tile([128, IS], F32, tag="ps", name="pr")
                nc.tensor.matmul(pr[:], lhsT=ones1[:], rhs=s2t[:], start=True, stop=True)
                s2rep = sb.tile([128, IS], BF16, tag="s2rep", name="s2rep")
                nc.vector.tensor_copy(s2rep[:], pr[:])

                # ---- layer-2 attention --------------------------------------
                agg2t = ps.tile([CLS + 1, 512], F32, tag="ps", name="agg2t")
                for jp in range(JT // 2):
                    el2 = wk.tile([128, 2 * IS], BF16, tag="el", name="el2", bufs=12)
                    for d in range(2):
                        j = 2 * jp + d
                        nc.vector._custom_dve(
                            GAT_EDGE2, out=el2[:, d * IS:(d + 1) * IS], in0=s2rep[:],
                            in1=mask_sb[j], s0=t2_sb[:, j:j + 1], s1=NEG, imm2=SLOPE)
                    p2 = wk.tile([128, 2 * IS], BF16, tag="p", name="p2", bufs=12)
                    nc.scalar.activation(p2[:], el2[:], AF.Exp)
                    for d in range(2):
                        j = 2 * jp + d
                        nc.tensor.matmul(
                            agg2t[:], lhsT=g2(j)[:, 0:CLS + 1],
                            rhs=p2[:, d * IS:(d + 1) * IS],
                            start=(j == 0), stop=(j == JT - 1),
                        )
                yt_sb = wk.tile([CLS + 1, 512], F32, tag="yt", name="yt")
                nc.scalar.copy(yt_sb[:], agg2t[:])
                nc.sync.dma_start(y[:, :], yt_sb[:])

    nc.compile()
    return nc


def _get_nc():
    global _NC_CACHE
    if _NC_CACHE is None:
        _NC_CACHE = build()
    return _NC_CACHE


def make_in_maps(x, adj_mat, W1, a1_src, a1_dst, W2, a2_src, a2_dst):
    x = np.asarray(x, dtype=np.float32)
    adj = np.asarray(adj_mat, dtype=bool)
    W1 = np.asarray(W1, dtype=np.float32)
    a1_src = np.asarray(a1_src, dtype=np.float32)
    a1_dst = np.asarray(a1_dst, dtype=np.float32)
    W2 = np.asarray(W2, dtype=np.float32)
    a2_src = np.asarray(a2_src, dtype=np.float32)
    a2_dst = np.asarray(a2_dst, dtype=np.float32)

    # host-side tiny precomputation (O(N*IN) matmuls with 8-col outputs)
    W1r = W1.astype(np.float64).reshape(IN, HEADS, HID)
    w1s = np.einsum("khf,f->kh", W1r, a1_src.astype(np.float64))
    w1d = np.einsum("khf,f->kh", W1r, a1_dst.astype(np.float64))
    xd = x.astype(np.float64)
    s1 = (xd @ w1s).astype(np.float32)          # [N, HEADS]
    t1 = (xd @ w1d).astype(np.float32)          # [N, HEADS]
    # W2 pre-scaled by 1/HEADS (head mean); aug cols: t2 | s2
    w2aug = np.concatenate(
        [W2, (W2.astype(np.float64) @ a2_dst.astype(np.float64))[:, None].astype(np.float32),
         (W2.astype(np.float64) @ a2_src.astype(np.float64))[:, None].astype(np.float32)],
        axis=1,
    )                                            # [HID, CLS+2]: g2 | t2 | s2
    mask_neg = (~adj).T.astype(np.uint8)                          # [N(j), N(i)]
    xt_all = np.ascontiguousarray(x.T).astype(BF)                  # [IN, N]
    w1_bf = W1.astype(BF)
    w2a_bf = w2aug.astype(BF)

    in_maps = []
    for c in range(NCORES):
        isl = slice(c * IS, (c + 1) * IS)
        srep_c = np.broadcast_to(
            np.ascontiguousarray(s1[isl].T).reshape(1, HEADS * IS), (128, HEADS * IS)
        ).astype(BF)
        in_maps.append({
            "xt": xt_all,
            "w1": w1_bf,
            "srep": np.ascontiguousarray(srep_c),
            "t1": t1,
            "maskt": np.ascontiguousarray(mask_neg[:, isl]),
            "w2a": w2a_bf,
        })

    return in_maps


def assemble_output(results, expect_reps=1):
    v = float(results[0]["ver"][0, 0])
    assert v == KVERSION * 1000 + expect_reps, (
        f"stale NEFF executed: ver={v}, want {KVERSION * 1000 + expect_reps}")
    outs = []
    for c in range(NCORES):
        raw = results[c]["y"]        # [CLS+1, IS]: rows 0:CLS unnorm, row CLS = Z
        outs.append((raw[0:CLS] / raw[CLS:CLS + 1]).T)
    return np.concatenate(outs, axis=0).astype(np.float32)


def kernel(x, adj_mat, W1, a1_src, a1_dst, W2, a2_src, a2_dst):
    in_maps = make_in_maps(x, adj_mat, W1, a1_src, a1_dst, W2, a2_src, a2_dst)
    global _last_in_maps
    _last_in_maps = in_maps
    nc = _get_nc()
    res = run_bass_kernel_spmd(nc, in_maps, core_ids=list(range(NCORES)))
    return assemble_output(res.results)
